# revision 8
# baseline (speedup 1.0000x reference)
"""AttentionPooling (PMA: one learnable seed query cross-attends each ragged
segment) as a Bass/Tile kernel on 8 Trainium2 NeuronCores.

Math restructuring (exact up to fp rounding):
  q  = LN(seed); qh = (q @ w_q.T + b_q) * 1/sqrt(DH)            (host, O(D^2))
  scores[t,h] = LN(x)[t] @ Wq[:,h] + const(h); the per-head const cancels in
                the segment softmax, so it is dropped. Wq[f,h] = sum_dh
                w_k[h*DH+dh,f]*qh[h,dh]. LN gamma folds into Wq / w_v,
                LN beta & b_v fold into an effective output bias.
  e  = exp(scores)           (no segment-max subtraction needed: |scores|<~2)
  pooled[b] = (sum_{t in b} e*v) / (sum_{t in b} e)
  out = pooled @ w_o.T + b_out_eff

Device dataflow per 128-token tile (token-major):
  DMA x -> bn_stats/bn_aggr -> rsqrt -> xhat=(x-m)*rinv (gpsimd)
  -> PE transpose (2x 128x128) -> PSUM->SBUF copies (ACT+DVE)
  -> PE matmul xhatT.T @ [w_v_folded | wq_folded]  (128,260) f32r
  -> ACT exp  -> DVE broadcast-mul e*v
  -> one-hot(event) = is_equal(iota, batch_local) (gpsimd)
  -> PE matmul accumulate one_hot.T @ [ev | e] into persistent PSUM (128,260)
Final per core: den guard, reciprocal, scale, transpose, out-proj matmul.

Sharding: 128 events per core (segment boundaries via searchsorted), tokens
padded to a common tile count; pad tokens get batch_local=-1 (one-hot row 0).
"""

import math
from contextlib import ExitStack

import numpy as np

import concourse.bacc as bacc
import concourse.mybir as mybir
import concourse.tile as tile
from concourse.bass_utils import run_bass_kernel_spmd

P = 128          # SBUF partitions
B = 1024         # events
D = 256          # embed dim
H = 4            # heads
DH = D // H
EPS = 1e-5
NCORES = 8
BC = B // NCORES  # events per core = 128
F32 = mybir.dt.float32
F32R = mybir.dt.float32r
AF = mybir.ActivationFunctionType
OP = mybir.AluOpType

def build_program(nt: int):
    nc = bacc.Bacc("TRN2", target_bir_lowering=False, debug=False,
                   num_devices=NCORES)

    x_d = nc.dram_tensor("x", [nt * P, D], F32, kind="ExternalInput")
    bl_d = nc.dram_tensor("bl", [P, nt], F32, kind="ExternalInput")
    wvq_d = nc.dram_tensor("wvq", [D, D + H], F32R, kind="ExternalInput")
    wot_d = nc.dram_tensor("wot", [D, D], F32R, kind="ExternalInput")
    bout_d = nc.dram_tensor("bout", [1, D], F32R, kind="ExternalInput")
    ident_d = nc.dram_tensor("ident", [P, P], F32R, kind="ExternalInput")
    ones_d = nc.dram_tensor("ones", [1, P], F32R, kind="ExternalInput")
    iota_d = nc.dram_tensor("iota", [P, P], F32, kind="ExternalInput")
    out_d = nc.dram_tensor("out", [P, D], F32, kind="ExternalOutput")

    with tile.TileContext(nc) as tc, ExitStack() as ctx:
        singles = ctx.enter_context(tc.tile_pool(name="singles", bufs=1))
        xpool = ctx.enter_context(tc.tile_pool(name="xpool", bufs=4))
        spool = ctx.enter_context(tc.tile_pool(name="spool", bufs=6))
        tpool = ctx.enter_context(tc.tile_pool(name="tpool", bufs=3))
        ppool = ctx.enter_context(tc.tile_pool(name="ppool", bufs=2, space="PSUM"))
        apool = ctx.enter_context(tc.tile_pool(name="apool", bufs=1, space="PSUM"))

        wvq_sb = singles.tile([P, 2, D + H], F32R)
        nc.sync.dma_start(wvq_sb[:, 0, :], wvq_d[0:P, :])
        nc.sync.dma_start(wvq_sb[:, 1, :], wvq_d[P:2 * P, :])
        wot_sb = singles.tile([P, 2, D], F32R)
        nc.sync.dma_start(wot_sb[:, 0, :], wot_d[0:P, :])
        nc.sync.dma_start(wot_sb[:, 1, :], wot_d[P:2 * P, :])
        bout_sb = singles.tile([1, D], F32R)
        nc.sync.dma_start(bout_sb, bout_d[:])
        ident_sb = singles.tile([P, P], F32R)
        nc.sync.dma_start(ident_sb, ident_d[:])
        iota_sb = singles.tile([P, P], F32)
        nc.sync.dma_start(iota_sb, iota_d[:])
        bl_sb = singles.tile([P, nt], F32)
        nc.sync.dma_start(bl_sb, bl_d[:])
        ones_sb = singles.tile([1, P], F32R)
        nc.sync.dma_start(ones_sb, ones_d[:])
        eps_sb = singles.tile([P, 1], F32)
        nc.vector.memset(eps_sb, EPS)

        acc = apool.tile([P, D + H], F32)

        for i in range(nt):
            x_t = xpool.tile([P, D], F32, tag="x")
            nc.sync.dma_start(x_t, x_d[i * P:(i + 1) * P, :])

            st6 = spool.tile([P, 6], F32, tag="st6")
            nc.vector.bn_stats(st6, x_t)
            mv = spool.tile([P, 2], F32, tag="mv")
            nc.vector.bn_aggr(mv, st6)
            rst = spool.tile([P, 1], F32, tag="rst")
            nc.scalar.activation(rst, mv[:, 1:2], AF.Sqrt, bias=eps_sb)
            rinv = spool.tile([P, 1], F32, tag="rinv")
            nc.vector.reciprocal(rinv, rst)

            xhat = tpool.tile([P, D], F32R, tag="xhat")
            nc.gpsimd.tensor_scalar(xhat, x_t, mv[:, 0:1], rinv,
                                    OP.subtract, OP.mult)

            xhT_ps = ppool.tile([P, 2, P], F32R, tag="xT_ps")
            nc.tensor.transpose(xhT_ps[:, 0, :], xhat[:, 0:P], ident_sb)
            nc.tensor.transpose(xhT_ps[:, 1, :], xhat[:, P:2 * P], ident_sb)
            xhT = tpool.tile([P, 2, P], F32R, tag="xhT")
            nc.scalar.copy(xhT[:, 0, :], xhT_ps[:, 0, :])
            nc.vector.tensor_copy(xhT[:, 1, :], xhT_ps[:, 1, :])

            vs_ps = ppool.tile([P, D + H], F32, tag="vs_ps")
            nc.tensor.matmul(vs_ps, lhsT=xhT[:, 0, :],
                             rhs=wvq_sb[:, 0, :], start=True, stop=False)
            nc.tensor.matmul(vs_ps, lhsT=xhT[:, 1, :],
                             rhs=wvq_sb[:, 1, :], start=False, stop=True)

            rhs_sb = tpool.tile([P, D + H], F32R, tag="rhs")
            nc.scalar.activation(rhs_sb[:, D:D + H], vs_ps[:, D:D + H], AF.Exp)
            nc.vector.tensor_tensor(
                out=rhs_sb[:, 0:D].rearrange("p (h d) -> p h d", h=H),
                in0=vs_ps[:, 0:D].rearrange("p (h d) -> p h d", h=H),
                in1=rhs_sb[:, D:D + H].to_broadcast((P, H, DH)),
                op=OP.mult)

            oh = tpool.tile([P, P], F32R, tag="oh")
            nc.gpsimd.tensor_scalar(oh, iota_sb, bl_sb[:, i:i + 1], None,
                                    OP.is_equal)

            nc.tensor.matmul(acc, lhsT=oh, rhs=rhs_sb,
                             start=(i == 0), stop=(i == nt - 1))

        # ---- finalization: divide by den, out-projection, bias ----
        den = acc[:, D:D + H]
        dz = spool.tile([P, H], F32, tag="dz")
        nc.vector.tensor_scalar(dz, den, 0.0, None, OP.is_equal)
        dg = spool.tile([P, H], F32, tag="dg")
        nc.vector.tensor_tensor(dg, den, dz, OP.add)
        rden = spool.tile([P, H], F32, tag="rden")
        nc.vector.reciprocal(rden, dg)

        pooled = tpool.tile([P, D], F32R, tag="pooled")
        nc.vector.tensor_tensor(
            out=pooled.rearrange("p (h d) -> p h d", h=H),
            in0=acc[:, 0:D].rearrange("p (h d) -> p h d", h=H),
            in1=rden.to_broadcast((P, H, DH)),
            op=OP.mult)

        pT_ps = ppool.tile([P, 2, P], F32R, tag="xT_ps")
        nc.tensor.transpose(pT_ps[:, 0, :], pooled[:, 0:P], ident_sb)
        nc.tensor.transpose(pT_ps[:, 1, :], pooled[:, P:2 * P], ident_sb)
        pT = tpool.tile([P, 2, P], F32R, tag="xhT")
        nc.scalar.copy(pT[:, 0, :], pT_ps[:, 0, :])
        nc.vector.tensor_copy(pT[:, 1, :], pT_ps[:, 1, :])

        out_ps = ppool.tile([P, D], F32, tag="vs_ps")
        nc.tensor.matmul(out_ps, lhsT=pT[:, 0, :],
                         rhs=wot_sb[:, 0, :], start=True, stop=False)
        nc.tensor.matmul(out_ps, lhsT=pT[:, 1, :],
                         rhs=wot_sb[:, 1, :], start=False, stop=False)
        nc.tensor.matmul(out_ps, lhsT=ones_sb, rhs=bout_sb,
                         start=False, stop=True)
        out_sb = tpool.tile([P, D], F32, tag="out")
        nc.vector.tensor_copy(out_sb, out_ps)
        nc.sync.dma_start(out_d[:], out_sb)

    nc.compile()
    return nc


def _prep_weights(seed, ln_q_w, ln_q_b, ln_k_w, ln_k_b,
                  w_q, b_q, w_k, b_k, w_v, b_v, w_o, b_o):
    s = seed[0, 0].astype(np.float32)
    m = s.mean()
    v = ((s - m) ** 2).mean()
    q = (s - m) / np.sqrt(v + EPS) * ln_q_w + ln_q_b
    qh = ((q @ w_q.T + b_q) * (1.0 / np.sqrt(DH))).reshape(H, DH)
    Wq = np.einsum('hdf,hd->fh', w_k.reshape(H, DH, D), qh)      # (D, H)
    wq_t = ln_k_w[:, None] * Wq                                   # (D, H)
    wv = ln_k_w[:, None] * w_v.T                                  # (D, D)
    WVQ = np.ascontiguousarray(
        np.concatenate([wv, wq_t], axis=1), dtype=np.float32)     # (D, D+H)
    cv = ln_k_b @ w_v.T + b_v                                     # (D,)
    woT = np.ascontiguousarray(w_o.T, dtype=np.float32)           # (D, D)
    bout = np.ascontiguousarray(
        (b_o + cv @ w_o.T)[None, :], dtype=np.float32)            # (1, D)
    return WVQ, woT, bout


def kernel(**inputs) -> np.ndarray:
    x = np.ascontiguousarray(np.asarray(inputs["x"], dtype=np.float32))
    batch = np.asarray(inputs["batch"]).astype(np.int64)
    WVQ, woT, bout = _prep_weights(
        *[np.asarray(inputs[k], dtype=np.float32) for k in
          ("seed", "ln_q_w", "ln_q_b", "ln_k_w", "ln_k_b",
           "w_q", "b_q", "w_k", "b_k", "w_v", "b_v", "w_o", "b_o")])

    bounds = np.searchsorted(batch, np.arange(0, B + 1, BC))
    counts = np.diff(bounds)
    nt = max(1, math.ceil(int(counts.max()) / P))
    ntok = nt * P

    ident = np.eye(P, dtype=np.float32)
    iota = np.tile(np.arange(P, dtype=np.float32), (P, 1))
    iota = np.ascontiguousarray(iota)

    in_maps = []
    for c in range(NCORES):
        s, e = int(bounds[c]), int(bounds[c + 1])
        n = e - s
        xc = np.zeros((ntok, D), np.float32)
        xc[:n] = x[s:e]
        bl = np.full((ntok,), -1.0, np.float32)
        bl[:n] = (batch[s:e] - c * BC).astype(np.float32)
        bl_t = np.ascontiguousarray(bl.reshape(nt, P).T)
        in_maps.append({"x": xc, "bl": bl_t, "wvq": WVQ, "wot": woT,
                        "bout": bout, "ident": ident, "iota": iota,
                        "ones": np.ones((1, P), np.float32)})

    nc = build_program(nt)
    global LAST_NC
    LAST_NC = nc
    res = run_bass_kernel_spmd(nc, in_maps, core_ids=list(range(NCORES)))
    out = np.concatenate([r["out"] for r in res.results], axis=0)
    return out.astype(np.float32)


if __name__ == "__main__":
    rng = np.random.default_rng(0)
    print("kernel module loaded")


# revision 17
# speedup vs baseline: 6.6792x; 6.6792x over previous
"""AttentionPooling (PMA: one learnable seed query cross-attends each ragged
segment) as a Bass/Tile kernel on 8 Trainium2 NeuronCores.

Math restructuring (exact up to fp rounding):
  q  = LN(seed); qh = (q @ w_q.T + b_q) * 1/sqrt(DH)            (host, O(D^2))
  scores[t,h] = LN(x)[t] @ Wq[:,h] + const(h); the per-head const cancels in
                the segment softmax, so it is dropped. LN gamma folds into
                Wq / w_v, LN beta & b_v fold into an effective output bias.
  LN applied as: xc = x - mean;  1/std factored out of the matmul and applied
  inside the exp (scale) and via er = e/std for the value weighting:
    e  = exp(s_u * rinv)         er = exp(s_u * rinv + ln(rinv))
    pooled[b] = (sum_t er * v_u) / (sum_t e)      (v_u = xc @ w_v_folded)
  out = pooled @ w_o.T + b_out_eff

Device dataflow (per 4-tile supertile of 512 tokens, bf16 compute path):
  SWDGE cast-load x->bf16 (128,4,256); bn_stats per tile; per-supertile
  negative-mean merge (DVE); per-64-tile group: variance merge + rinv and
  ln(rinv) via batched Ln/Exp on ACT (the only non-Exp LUT loads happen
  once per group); xc = x - m (DVE); PE bf16 transposes -> PSUM; one
  batched DVE copy -> SBUF; 2 bf16 matmuls xcT.T @ [w_v|wq] -> PSUM;
  per tile ACT Exp e (den cols) and er; batched DVE broadcast-mul er*v;
  one-hot(event) = is_equal(iota, batch_local); f32r matmul accumulating
  one_hot.T @ [ev | e] into a persistent PSUM accumulator (segment-sum).
Final per core: den guard, reciprocal, scale, transpose, out-proj matmul.

Sharding: 128 events per core (segment boundaries via searchsorted), tokens
padded to a common tile count; pad tokens get batch_local=-1 (one-hot row 0).
"""

import math
from contextlib import ExitStack

import ml_dtypes
import numpy as np

import concourse.bacc as bacc
import concourse.mybir as mybir
import concourse.tile as tile
from concourse.bass_utils import run_bass_kernel_spmd

P = 128          # SBUF partitions
B = 1024         # events
D = 256          # embed dim
H = 4            # heads
DH = D // H
EPS = 1e-5
NCORES = 8
BC = B // NCORES  # events per core = 128
ST = 4            # tiles per supertile (DMA/stats batch)
GT = 64           # tiles per rsqrt group
F32 = mybir.dt.float32
F32R = mybir.dt.float32r
BF16 = mybir.dt.bfloat16
AF = mybir.ActivationFunctionType
OP = mybir.AluOpType


def build_program(nt: int):
    assert nt % ST == 0
    nc = bacc.Bacc("TRN2", target_bir_lowering=False, debug=False,
                   num_devices=NCORES)

    x_d = nc.dram_tensor("x", [nt * P, D], F32, kind="ExternalInput")
    bl_d = nc.dram_tensor("bl", [P, nt], F32, kind="ExternalInput")
    wvq_d = nc.dram_tensor("wvq", [D, D + H], BF16, kind="ExternalInput")
    wot_d = nc.dram_tensor("wot", [D, D], F32R, kind="ExternalInput")
    bout_d = nc.dram_tensor("bout", [1, D], F32R, kind="ExternalInput")
    identb_d = nc.dram_tensor("identb", [P, P], BF16, kind="ExternalInput")
    ident_d = nc.dram_tensor("ident", [P, P], F32R, kind="ExternalInput")
    ones_d = nc.dram_tensor("ones", [1, P], F32R, kind="ExternalInput")
    iota_d = nc.dram_tensor("iota", [P, P], F32, kind="ExternalInput")
    out_d = nc.dram_tensor("out", [P, D], F32, kind="ExternalOutput")

    with tile.TileContext(nc) as tc, ExitStack() as ctx:
        singles = ctx.enter_context(tc.tile_pool(name="singles", bufs=1))
        xpool = ctx.enter_context(tc.tile_pool(name="xpool", bufs=22))
        cpool = ctx.enter_context(tc.tile_pool(name="cpool", bufs=4))
        tpool = ctx.enter_context(tc.tile_pool(name="tpool", bufs=4))
        rpool = ctx.enter_context(tc.tile_pool(name="rpool", bufs=6))
        gpool = ctx.enter_context(tc.tile_pool(name="gpool", bufs=2))
        spool = ctx.enter_context(tc.tile_pool(name="spool", bufs=4))
        ppool = ctx.enter_context(tc.tile_pool(name="ppool", bufs=2, space="PSUM"))
        qpool = ctx.enter_context(tc.tile_pool(name="qpool", bufs=2, space="PSUM"))
        apool = ctx.enter_context(tc.tile_pool(name="apool", bufs=1, space="PSUM"))

        wvq_sb = singles.tile([P, 2, D + H], BF16)
        nc.sync.dma_start(wvq_sb[:, 0, :], wvq_d[0:P, :])
        nc.sync.dma_start(wvq_sb[:, 1, :], wvq_d[P:2 * P, :])
        wot_sb = singles.tile([P, 2, D], F32R)
        nc.sync.dma_start(wot_sb[:, 0, :], wot_d[0:P, :])
        nc.sync.dma_start(wot_sb[:, 1, :], wot_d[P:2 * P, :])
        bout_sb = singles.tile([1, D], F32R)
        nc.sync.dma_start(bout_sb, bout_d[:])
        identb_sb = singles.tile([P, P], BF16)
        nc.sync.dma_start(identb_sb, identb_d[:])
        ident_sb = singles.tile([P, P], F32R)
        nc.sync.dma_start(ident_sb, ident_d[:])
        ones_sb = singles.tile([1, P], F32R)
        nc.sync.dma_start(ones_sb, ones_d[:])
        iota_sb = singles.tile([P, P], F32)
        nc.sync.dma_start(iota_sb, iota_d[:])
        bl_sb = singles.tile([P, nt], F32)
        nc.sync.dma_start(bl_sb, bl_d[:])
        eps_sb = singles.tile([P, 1], F32)
        nc.vector.memset(eps_sb, EPS)

        acc = apool.tile([P, D + H], F32, tag="acc")

        for g0 in range(0, nt, GT):
            gsz = min(GT, nt - g0)
            nst = gsz // ST
            st6g = gpool.tile([P, gsz, 6], F32, tag="st6g")
            nmg = gpool.tile([P, gsz], F32, tag="nmg")     # negative mean
            varg = gpool.tile([P, gsz], F32, tag="varg")
            rinvg = gpool.tile([P, gsz], F32, tag="rinvg")
            lnrg = gpool.tile([P, gsz], F32, tag="lnrg")
            tmpg = gpool.tile([P, gsz], F32, tag="tmpg")

            # ---- phase A: load + stats + negative supertile mean ----
            x4s = []
            for s in range(nst):
                i0 = g0 + s * ST
                x4 = xpool.tile([P, ST, D], F32, tag="x4")
                x4s.append(x4)
                nc.sync.dma_start(
                    out=x4,
                    in_=x_d[i0 * P:(i0 + ST) * P, :].rearrange(
                        "(k p) f -> p k f", p=P))
                sl = slice(s * ST, (s + 1) * ST)
                for k in range(ST):
                    nc.vector.bn_stats(st6g[:, s * ST + k, :], x4[:, k, :])
                nc.vector.tensor_tensor(nmg[:, sl], st6g[:, sl, 1],
                                        st6g[:, sl, 4], OP.add)
                nc.vector.tensor_scalar(nmg[:, sl], nmg[:, sl], -0.5, None,
                                        OP.mult)

            # ---- phase B: group variance; rinv = exp(-0.5*ln(var+eps)) ----
            nc.vector.tensor_tensor(tmpg, st6g[:, :, 1], st6g[:, :, 4],
                                    OP.subtract)
            nc.vector.tensor_tensor(tmpg, tmpg, tmpg, OP.mult)
            nc.vector.tensor_scalar(tmpg, tmpg, 0.25, None, OP.mult)
            nc.vector.tensor_tensor(varg, st6g[:, :, 2], st6g[:, :, 5],
                                    OP.add)
            nc.vector.tensor_scalar(varg, varg, 1.0 / D, None, OP.mult)
            nc.vector.tensor_tensor(varg, varg, tmpg, OP.add)
            nc.scalar.activation(lnrg, varg, AF.Ln, bias=eps_sb)
            nc.vector.tensor_scalar(lnrg, lnrg, -0.5, None, OP.mult)
            nc.scalar.activation(rinvg, lnrg, AF.Exp)

            # ---- phase C: center, transpose, matmul, softmax, pool ----
            for s in range(nst):
                x4 = x4s[s]
                xc4 = cpool.tile([P, ST, D], BF16, tag="xc4")
                for k in range(ST):
                    gi = s * ST + k
                    nc.vector.tensor_scalar(xc4[:, k, :], x4[:, k, :],
                                            nmg[:, gi:gi + 1], None, OP.add)
                xcT_ps = ppool.tile([P, 2 * ST, P], BF16, tag="xcT_ps")
                for k in range(ST):
                    nc.tensor.transpose(xcT_ps[:, 2 * k, :],
                                        xc4[:, k, 0:P], identb_sb)
                    nc.tensor.transpose(xcT_ps[:, 2 * k + 1, :],
                                        xc4[:, k, P:2 * P], identb_sb)
                xcT = tpool.tile([P, 2 * ST, P], BF16, tag="xcT")
                nc.vector.tensor_copy(xcT, xcT_ps)

                for kk in range(ST // 2):
                    vs2 = qpool.tile([P, 2, 512], F32, tag="vs2")
                    er2 = rpool.tile([P, 2, H], F32, tag="er2")
                    rhs2 = rpool.tile([P, 2, 264], F32R, tag="rhs2")
                    for j2 in range(2):
                        k = kk * 2 + j2
                        gi = s * ST + k
                        nc.tensor.matmul(vs2[:, j2, 0:D + H],
                                         lhsT=xcT[:, 2 * k, :],
                                         rhs=wvq_sb[:, 0, :],
                                         start=True, stop=False)
                        nc.tensor.matmul(vs2[:, j2, 0:D + H],
                                         lhsT=xcT[:, 2 * k + 1, :],
                                         rhs=wvq_sb[:, 1, :],
                                         start=False, stop=True)
                        nc.scalar.activation(rhs2[:, j2, D:D + H],
                                             vs2[:, j2, D:D + H], AF.Exp,
                                             scale=rinvg[:, gi:gi + 1])
                        nc.scalar.activation(er2[:, j2, :],
                                             vs2[:, j2, D:D + H], AF.Exp,
                                             scale=rinvg[:, gi:gi + 1],
                                             bias=lnrg[:, gi:gi + 1])
                    nc.vector.tensor_tensor(
                        out=rhs2[:, :, 0:D].rearrange(
                            "p a (h d) -> p a h d", h=H),
                        in0=vs2[:, :, 0:D].rearrange(
                            "p a (h d) -> p a h d", h=H),
                        in1=er2.to_broadcast((P, 2, H, DH)),
                        op=OP.mult)
                    for j2 in range(2):
                        k = kk * 2 + j2
                        idx = g0 + s * ST + k
                        oh = rpool.tile([P, P], F32R, tag="oh")
                        nc.vector.tensor_scalar(oh, iota_sb,
                                                bl_sb[:, idx:idx + 1], None,
                                                OP.is_equal)
                        nc.tensor.matmul(acc, lhsT=oh,
                                         rhs=rhs2[:, j2, 0:D + H],
                                         start=(idx == 0),
                                         stop=(idx == nt - 1))

        # ---- finalization: divide by den, out-projection, bias ----
        den = acc[:, D:D + H]
        dz = spool.tile([P, H], F32, tag="dz")
        nc.vector.tensor_scalar(dz, den, 0.0, None, OP.is_equal)
        dg = spool.tile([P, H], F32, tag="dg")
        nc.vector.tensor_tensor(dg, den, dz, OP.add)
        rden = spool.tile([P, H], F32, tag="rden")
        nc.vector.reciprocal(rden, dg)

        pooled = spool.tile([P, D], F32R, tag="pooled")
        nc.vector.tensor_tensor(
            out=pooled.rearrange("p (h d) -> p h d", h=H),
            in0=acc[:, 0:D].rearrange("p (h d) -> p h d", h=H),
            in1=rden.to_broadcast((P, H, DH)),
            op=OP.mult)

        pT_ps = ppool.tile([P, 2, P], F32R, tag="xcT_ps")
        nc.tensor.transpose(pT_ps[:, 0, :], pooled[:, 0:P], ident_sb)
        nc.tensor.transpose(pT_ps[:, 1, :], pooled[:, P:2 * P], ident_sb)
        pT = spool.tile([P, 2, P], F32R, tag="pT")
        nc.vector.tensor_copy(pT[:, 0, :], pT_ps[:, 0, :])
        nc.vector.tensor_copy(pT[:, 1, :], pT_ps[:, 1, :])

        out_ps = ppool.tile([P, D], F32, tag="xcT_ps")
        nc.tensor.matmul(out_ps, lhsT=pT[:, 0, :],
                         rhs=wot_sb[:, 0, :], start=True, stop=False)
        nc.tensor.matmul(out_ps, lhsT=pT[:, 1, :],
                         rhs=wot_sb[:, 1, :], start=False, stop=False)
        nc.tensor.matmul(out_ps, lhsT=ones_sb, rhs=bout_sb,
                         start=False, stop=True)
        out_sb = spool.tile([P, D], F32, tag="out")
        nc.vector.tensor_copy(out_sb, out_ps)
        nc.sync.dma_start(out_d[:], out_sb)

    nc.compile()
    return nc


def _prep_weights(seed, ln_q_w, ln_q_b, ln_k_w, ln_k_b,
                  w_q, b_q, w_k, b_k, w_v, b_v, w_o, b_o):
    s = seed[0, 0].astype(np.float32)
    m = s.mean()
    v = ((s - m) ** 2).mean()
    q = (s - m) / np.sqrt(v + EPS) * ln_q_w + ln_q_b
    qh = ((q @ w_q.T + b_q) * (1.0 / np.sqrt(DH))).reshape(H, DH)
    Wq = np.einsum('hdf,hd->fh', w_k.reshape(H, DH, D), qh)      # (D, H)
    wq_t = ln_k_w[:, None] * Wq                                   # (D, H)
    wv = ln_k_w[:, None] * w_v.T                                  # (D, D)
    WVQ = np.ascontiguousarray(
        np.concatenate([wv, wq_t], axis=1), dtype=np.float32)     # (D, D+H)
    cv = ln_k_b @ w_v.T + b_v                                     # (D,)
    woT = np.ascontiguousarray(w_o.T, dtype=np.float32)           # (D, D)
    bout = np.ascontiguousarray(
        (b_o + cv @ w_o.T)[None, :], dtype=np.float32)            # (1, D)
    return WVQ, woT, bout


def kernel(**inputs) -> np.ndarray:
    x = np.ascontiguousarray(np.asarray(inputs["x"], dtype=np.float32))
    batch = np.asarray(inputs["batch"]).astype(np.int64)
    WVQ, woT, bout = _prep_weights(
        *[np.asarray(inputs[k], dtype=np.float32) for k in
          ("seed", "ln_q_w", "ln_q_b", "ln_k_w", "ln_k_b",
           "w_q", "b_q", "w_k", "b_k", "w_v", "b_v", "w_o", "b_o")])

    bounds = np.searchsorted(batch, np.arange(0, B + 1, BC))
    counts = np.diff(bounds)
    nt = max(1, math.ceil(int(counts.max()) / P))
    nt = ((nt + ST - 1) // ST) * ST
    ntok = nt * P

    ident = np.eye(P, dtype=np.float32)
    iota = np.ascontiguousarray(
        np.tile(np.arange(P, dtype=np.float32), (P, 1)))
    wvq_bf = WVQ.astype(ml_dtypes.bfloat16)

    in_maps = []
    for c in range(NCORES):
        s, e = int(bounds[c]), int(bounds[c + 1])
        n = e - s
        xc = np.zeros((ntok, D), np.float32)
        xc[:n] = x[s:e]
        bl = np.full((ntok,), -1.0, np.float32)
        bl[:n] = (batch[s:e] - c * BC).astype(np.float32)
        bl_t = np.ascontiguousarray(bl.reshape(nt, P).T)
        in_maps.append({"x": xc, "bl": bl_t, "wvq": wvq_bf, "wot": woT,
                        "bout": bout, "ident": ident,
                        "identb": ident.astype(ml_dtypes.bfloat16),
                        "iota": iota,
                        "ones": np.ones((1, P), np.float32)})

    nc = build_program(nt)
    global LAST_NC
    LAST_NC = nc
    res = run_bass_kernel_spmd(nc, in_maps, core_ids=list(range(NCORES)))
    out = np.concatenate([r["out"] for r in res.results], axis=0)
    return out.astype(np.float32)


# revision 18
# speedup vs baseline: 6.7127x; 1.0050x over previous
"""AttentionPooling (PMA: one learnable seed query cross-attends each ragged
segment) as a Bass/Tile kernel on 8 Trainium2 NeuronCores.

Math restructuring (exact up to fp rounding):
  q  = LN(seed); qh = (q @ w_q.T + b_q) * 1/sqrt(DH)            (host, O(D^2))
  scores[t,h] = LN(x)[t] @ Wq[:,h] + const(h); the per-head const cancels in
                the segment softmax, so it is dropped. LN gamma folds into
                Wq / w_v, LN beta & b_v fold into an effective output bias.
  LN applied as: xc = x - mean;  1/std factored out of the matmul and applied
  inside the exp (scale) and via er = e/std for the value weighting:
    e  = exp(s_u * rinv)         er = exp(s_u * rinv + ln(rinv))
    pooled[b] = (sum_t er * v_u) / (sum_t e)      (v_u = xc @ w_v_folded)
  out = pooled @ w_o.T + b_out_eff

Device dataflow (per 4-tile supertile of 512 tokens, bf16 compute path):
  SWDGE cast-load x->bf16 (128,4,256); bn_stats per tile; per-supertile
  negative-mean merge (DVE); per-64-tile group: variance merge + rinv and
  ln(rinv) via batched Ln/Exp on ACT (the only non-Exp LUT loads happen
  once per group); xc = x - m (DVE); PE bf16 transposes -> PSUM; one
  batched DVE copy -> SBUF; 2 bf16 matmuls xcT.T @ [w_v|wq] -> PSUM;
  per tile ACT Exp e (den cols) and er; batched DVE broadcast-mul er*v;
  one-hot(event) = is_equal(iota, batch_local); f32r matmul accumulating
  one_hot.T @ [ev | e] into a persistent PSUM accumulator (segment-sum).
Final per core: den guard, reciprocal, scale, transpose, out-proj matmul.

Sharding: 128 events per core (segment boundaries via searchsorted), tokens
padded to a common tile count; pad tokens get batch_local=-1 (one-hot row 0).
"""

import math
from contextlib import ExitStack

import ml_dtypes
import numpy as np

import concourse.bacc as bacc
import concourse.mybir as mybir
import concourse.tile as tile
from concourse.bass_utils import run_bass_kernel_spmd

P = 128          # SBUF partitions
B = 1024         # events
D = 256          # embed dim
H = 4            # heads
DH = D // H
EPS = 1e-5
NCORES = 8
BC = B // NCORES  # events per core = 128
ST = 4            # tiles per supertile (DMA/stats batch)
GT = 64           # tiles per rsqrt group
F32 = mybir.dt.float32
F32R = mybir.dt.float32r
BF16 = mybir.dt.bfloat16
AF = mybir.ActivationFunctionType
OP = mybir.AluOpType


def build_program(nt: int):
    assert nt % ST == 0
    nc = bacc.Bacc("TRN2", target_bir_lowering=False, debug=False,
                   num_devices=NCORES)

    x_d = nc.dram_tensor("x", [nt * P, D], F32, kind="ExternalInput")
    bl_d = nc.dram_tensor("bl", [P, nt], F32, kind="ExternalInput")
    wvq_d = nc.dram_tensor("wvq", [D, D + H], BF16, kind="ExternalInput")
    wot_d = nc.dram_tensor("wot", [D, D], F32R, kind="ExternalInput")
    bout_d = nc.dram_tensor("bout", [1, D], F32R, kind="ExternalInput")
    identb_d = nc.dram_tensor("identb", [P, P], BF16, kind="ExternalInput")
    ident_d = nc.dram_tensor("ident", [P, P], F32R, kind="ExternalInput")
    ones_d = nc.dram_tensor("ones", [1, P], F32R, kind="ExternalInput")
    iota_d = nc.dram_tensor("iota", [P, P], F32, kind="ExternalInput")
    out_d = nc.dram_tensor("out", [P, D], F32, kind="ExternalOutput")

    with tile.TileContext(nc) as tc, ExitStack() as ctx:
        singles = ctx.enter_context(tc.tile_pool(name="singles", bufs=1))
        xpool = ctx.enter_context(tc.tile_pool(name="xpool", bufs=22))
        cpool = ctx.enter_context(tc.tile_pool(name="cpool", bufs=6))
        tpool = ctx.enter_context(tc.tile_pool(name="tpool", bufs=6))
        rpool = ctx.enter_context(tc.tile_pool(name="rpool", bufs=10))
        gpool = ctx.enter_context(tc.tile_pool(name="gpool", bufs=2))
        spool = ctx.enter_context(tc.tile_pool(name="spool", bufs=4))
        ppool = ctx.enter_context(tc.tile_pool(name="ppool", bufs=2, space="PSUM"))
        qpool = ctx.enter_context(tc.tile_pool(name="qpool", bufs=2, space="PSUM"))
        apool = ctx.enter_context(tc.tile_pool(name="apool", bufs=1, space="PSUM"))

        wvq_sb = singles.tile([P, 2, D + H], BF16)
        nc.sync.dma_start(wvq_sb[:, 0, :], wvq_d[0:P, :])
        nc.sync.dma_start(wvq_sb[:, 1, :], wvq_d[P:2 * P, :])
        wot_sb = singles.tile([P, 2, D], F32R)
        nc.sync.dma_start(wot_sb[:, 0, :], wot_d[0:P, :])
        nc.sync.dma_start(wot_sb[:, 1, :], wot_d[P:2 * P, :])
        bout_sb = singles.tile([1, D], F32R)
        nc.sync.dma_start(bout_sb, bout_d[:])
        identb_sb = singles.tile([P, P], BF16)
        nc.sync.dma_start(identb_sb, identb_d[:])
        ident_sb = singles.tile([P, P], F32R)
        nc.sync.dma_start(ident_sb, ident_d[:])
        ones_sb = singles.tile([1, P], F32R)
        nc.sync.dma_start(ones_sb, ones_d[:])
        iota_sb = singles.tile([P, P], F32)
        nc.sync.dma_start(iota_sb, iota_d[:])
        bl_sb = singles.tile([P, nt], F32)
        nc.sync.dma_start(bl_sb, bl_d[:])
        eps_sb = singles.tile([P, 1], F32)
        nc.vector.memset(eps_sb, EPS)

        acc = apool.tile([P, D + H], F32, tag="acc")

        for g0 in range(0, nt, GT):
            gsz = min(GT, nt - g0)
            nst = gsz // ST
            st6g = gpool.tile([P, gsz, 6], F32, tag="st6g")
            nmg = gpool.tile([P, gsz], F32, tag="nmg")     # negative mean
            varg = gpool.tile([P, gsz], F32, tag="varg")
            rinvg = gpool.tile([P, gsz], F32, tag="rinvg")
            lnrg = gpool.tile([P, gsz], F32, tag="lnrg")
            tmpg = gpool.tile([P, gsz], F32, tag="tmpg")

            # ---- phase A: load + stats + negative supertile mean ----
            x4s = []
            for s in range(nst):
                i0 = g0 + s * ST
                x4 = xpool.tile([P, ST, D], F32, tag="x4")
                x4s.append(x4)
                nc.sync.dma_start(
                    out=x4,
                    in_=x_d[i0 * P:(i0 + ST) * P, :].rearrange(
                        "(k p) f -> p k f", p=P))
                sl = slice(s * ST, (s + 1) * ST)
                for k in range(ST):
                    nc.vector.bn_stats(st6g[:, s * ST + k, :], x4[:, k, :])
                nc.vector.tensor_tensor(nmg[:, sl], st6g[:, sl, 1],
                                        st6g[:, sl, 4], OP.add)
                nc.vector.tensor_scalar(nmg[:, sl], nmg[:, sl], -0.5, None,
                                        OP.mult)

            # ---- phase B: group variance; rinv = exp(-0.5*ln(var+eps)) ----
            nc.vector.tensor_tensor(tmpg, st6g[:, :, 1], st6g[:, :, 4],
                                    OP.subtract)
            nc.vector.tensor_tensor(tmpg, tmpg, tmpg, OP.mult)
            nc.vector.tensor_scalar(tmpg, tmpg, 0.25, None, OP.mult)
            nc.vector.tensor_tensor(varg, st6g[:, :, 2], st6g[:, :, 5],
                                    OP.add)
            nc.vector.tensor_scalar(varg, varg, 1.0 / D, None, OP.mult)
            nc.vector.tensor_tensor(varg, varg, tmpg, OP.add)
            nc.scalar.activation(lnrg, varg, AF.Ln, bias=eps_sb)
            nc.vector.tensor_scalar(lnrg, lnrg, -0.5, None, OP.mult)
            nc.scalar.activation(rinvg, lnrg, AF.Exp)

            # ---- phase C: center, transpose, matmul, softmax, pool ----
            for s in range(nst):
                x4 = x4s[s]
                xc4 = cpool.tile([P, ST, D], BF16, tag="xc4")
                for k in range(ST):
                    gi = s * ST + k
                    nc.vector.tensor_scalar(xc4[:, k, :], x4[:, k, :],
                                            nmg[:, gi:gi + 1], None, OP.add)
                xcT_ps = ppool.tile([P, 2 * ST, P], BF16, tag="xcT_ps")
                for k in range(ST):
                    nc.tensor.transpose(xcT_ps[:, 2 * k, :],
                                        xc4[:, k, 0:P], identb_sb)
                    nc.tensor.transpose(xcT_ps[:, 2 * k + 1, :],
                                        xc4[:, k, P:2 * P], identb_sb)
                xcT = tpool.tile([P, 2 * ST, P], BF16, tag="xcT")
                nc.vector.tensor_copy(xcT, xcT_ps)

                for kk in range(ST // 2):
                    vs2 = qpool.tile([P, 2, 512], F32, tag="vs2")
                    er2 = rpool.tile([P, 2, H], F32, tag="er2")
                    rhs2 = rpool.tile([P, 2, 264], F32R, tag="rhs2")
                    for j2 in range(2):
                        k = kk * 2 + j2
                        gi = s * ST + k
                        nc.tensor.matmul(vs2[:, j2, 0:D + H],
                                         lhsT=xcT[:, 2 * k, :],
                                         rhs=wvq_sb[:, 0, :],
                                         start=True, stop=False)
                        nc.tensor.matmul(vs2[:, j2, 0:D + H],
                                         lhsT=xcT[:, 2 * k + 1, :],
                                         rhs=wvq_sb[:, 1, :],
                                         start=False, stop=True)
                        nc.scalar.activation(rhs2[:, j2, D:D + H],
                                             vs2[:, j2, D:D + H], AF.Exp,
                                             scale=rinvg[:, gi:gi + 1])
                        nc.scalar.activation(er2[:, j2, :],
                                             vs2[:, j2, D:D + H], AF.Exp,
                                             scale=rinvg[:, gi:gi + 1],
                                             bias=lnrg[:, gi:gi + 1])
                    nc.vector.tensor_tensor(
                        out=rhs2[:, :, 0:D].rearrange(
                            "p a (h d) -> p a h d", h=H),
                        in0=vs2[:, :, 0:D].rearrange(
                            "p a (h d) -> p a h d", h=H),
                        in1=er2.to_broadcast((P, 2, H, DH)),
                        op=OP.mult)
                    for j2 in range(2):
                        k = kk * 2 + j2
                        idx = g0 + s * ST + k
                        oh = rpool.tile([P, P], F32R, tag="oh")
                        nc.vector.tensor_scalar(oh, iota_sb,
                                                bl_sb[:, idx:idx + 1], None,
                                                OP.is_equal)
                        nc.tensor.matmul(acc, lhsT=oh,
                                         rhs=rhs2[:, j2, 0:D + H],
                                         start=(idx == 0),
                                         stop=(idx == nt - 1))

        # ---- finalization: divide by den, out-projection, bias ----
        den = acc[:, D:D + H]
        dz = spool.tile([P, H], F32, tag="dz")
        nc.vector.tensor_scalar(dz, den, 0.0, None, OP.is_equal)
        dg = spool.tile([P, H], F32, tag="dg")
        nc.vector.tensor_tensor(dg, den, dz, OP.add)
        rden = spool.tile([P, H], F32, tag="rden")
        nc.vector.reciprocal(rden, dg)

        pooled = spool.tile([P, D], F32R, tag="pooled")
        nc.vector.tensor_tensor(
            out=pooled.rearrange("p (h d) -> p h d", h=H),
            in0=acc[:, 0:D].rearrange("p (h d) -> p h d", h=H),
            in1=rden.to_broadcast((P, H, DH)),
            op=OP.mult)

        pT_ps = ppool.tile([P, 2, P], F32R, tag="xcT_ps")
        nc.tensor.transpose(pT_ps[:, 0, :], pooled[:, 0:P], ident_sb)
        nc.tensor.transpose(pT_ps[:, 1, :], pooled[:, P:2 * P], ident_sb)
        pT = spool.tile([P, 2, P], F32R, tag="pT")
        nc.vector.tensor_copy(pT[:, 0, :], pT_ps[:, 0, :])
        nc.vector.tensor_copy(pT[:, 1, :], pT_ps[:, 1, :])

        out_ps = ppool.tile([P, D], F32, tag="xcT_ps")
        nc.tensor.matmul(out_ps, lhsT=pT[:, 0, :],
                         rhs=wot_sb[:, 0, :], start=True, stop=False)
        nc.tensor.matmul(out_ps, lhsT=pT[:, 1, :],
                         rhs=wot_sb[:, 1, :], start=False, stop=False)
        nc.tensor.matmul(out_ps, lhsT=ones_sb, rhs=bout_sb,
                         start=False, stop=True)
        out_sb = spool.tile([P, D], F32, tag="out")
        nc.vector.tensor_copy(out_sb, out_ps)
        nc.sync.dma_start(out_d[:], out_sb)

    nc.compile()
    return nc


def _prep_weights(seed, ln_q_w, ln_q_b, ln_k_w, ln_k_b,
                  w_q, b_q, w_k, b_k, w_v, b_v, w_o, b_o):
    s = seed[0, 0].astype(np.float32)
    m = s.mean()
    v = ((s - m) ** 2).mean()
    q = (s - m) / np.sqrt(v + EPS) * ln_q_w + ln_q_b
    qh = ((q @ w_q.T + b_q) * (1.0 / np.sqrt(DH))).reshape(H, DH)
    Wq = np.einsum('hdf,hd->fh', w_k.reshape(H, DH, D), qh)      # (D, H)
    wq_t = ln_k_w[:, None] * Wq                                   # (D, H)
    wv = ln_k_w[:, None] * w_v.T                                  # (D, D)
    WVQ = np.ascontiguousarray(
        np.concatenate([wv, wq_t], axis=1), dtype=np.float32)     # (D, D+H)
    cv = ln_k_b @ w_v.T + b_v                                     # (D,)
    woT = np.ascontiguousarray(w_o.T, dtype=np.float32)           # (D, D)
    bout = np.ascontiguousarray(
        (b_o + cv @ w_o.T)[None, :], dtype=np.float32)            # (1, D)
    return WVQ, woT, bout


def kernel(**inputs) -> np.ndarray:
    x = np.ascontiguousarray(np.asarray(inputs["x"], dtype=np.float32))
    batch = np.asarray(inputs["batch"]).astype(np.int64)
    WVQ, woT, bout = _prep_weights(
        *[np.asarray(inputs[k], dtype=np.float32) for k in
          ("seed", "ln_q_w", "ln_q_b", "ln_k_w", "ln_k_b",
           "w_q", "b_q", "w_k", "b_k", "w_v", "b_v", "w_o", "b_o")])

    bounds = np.searchsorted(batch, np.arange(0, B + 1, BC))
    counts = np.diff(bounds)
    nt = max(1, math.ceil(int(counts.max()) / P))
    nt = ((nt + ST - 1) // ST) * ST
    ntok = nt * P

    ident = np.eye(P, dtype=np.float32)
    iota = np.ascontiguousarray(
        np.tile(np.arange(P, dtype=np.float32), (P, 1)))
    wvq_bf = WVQ.astype(ml_dtypes.bfloat16)

    in_maps = []
    for c in range(NCORES):
        s, e = int(bounds[c]), int(bounds[c + 1])
        n = e - s
        xc = np.zeros((ntok, D), np.float32)
        xc[:n] = x[s:e]
        bl = np.full((ntok,), -1.0, np.float32)
        bl[:n] = (batch[s:e] - c * BC).astype(np.float32)
        bl_t = np.ascontiguousarray(bl.reshape(nt, P).T)
        in_maps.append({"x": xc, "bl": bl_t, "wvq": wvq_bf, "wot": woT,
                        "bout": bout, "ident": ident,
                        "identb": ident.astype(ml_dtypes.bfloat16),
                        "iota": iota,
                        "ones": np.ones((1, P), np.float32)})

    nc = build_program(nt)
    global LAST_NC
    LAST_NC = nc
    res = run_bass_kernel_spmd(nc, in_maps, core_ids=list(range(NCORES)))
    out = np.concatenate([r["out"] for r in res.results], axis=0)
    return out.astype(np.float32)
